# revision 9
# baseline (speedup 1.0000x reference)
"""CFG sub-AST expression combiner (segment-softmax scatter attention) on 8 trn2 cores.

Strategy: sort edges by segment (PDG node) on host; give each core a contiguous
range of segments so every segment's softmax is core-local (no collectives).
Host folds Wq/Wk into a per-segment vector table C = A @ (Wq Wk^T)/sqrt(d), so
the device only needs: gather value rows -> scores = V @ C_window^T (PE) ->
exp (ACT) -> mask (DVE) -> PV + denominator matmuls (PE, PSUM-accumulated) ->
divide -> project with Wo.
"""

import math

import numpy as np
import ml_dtypes

import concourse.bass as bass
from concourse import bacc
import concourse.mybir as mybir
from concourse.bass import IndirectOffsetOnAxis
from concourse.bass_types import AP
from concourse.tile import TileContext, add_dep_helper
from concourse import bass_utils

BF16 = ml_dtypes.bfloat16
N_CORES = 8
D = 128          # feature dim
H = 8            # heads
W = 32           # segment window width (output columns per score matmul)
P = 128          # edge slots per tile (partition dim)
F32 = mybir.dt.float32
BF = mybir.dt.bfloat16
I32 = mybir.dt.int32


def _build_nc(NW, T_w, n_tbl, comb):
    """One SPMD program for all cores. NW windows of W segments; each window
    owns T_w tiles of P edge slots."""
    S_pad = NW * W
    nc = bacc.Bacc("TRN2", target_bir_lowering=False)

    tbl = nc.dram_tensor("tbl", [n_tbl, D], BF, kind="ExternalInput")
    gidx = nc.dram_tensor("gidx", [P, NW * T_w], I32, kind="ExternalInput")
    cc = nc.dram_tensor("cc", [D, NW * H * W], BF, kind="ExternalInput")
    msk = nc.dram_tensor("msk", [P, NW * T_w * W], BF, kind="ExternalInput")
    wo = nc.dram_tensor("wo", [D, H * comb], BF, kind="ExternalInput")
    bo = nc.dram_tensor("bo", [comb, 1], F32, kind="ExternalInput")
    ident = nc.dram_tensor("ident", [P, P], BF, kind="ExternalInput")
    out = nc.dram_tensor("out", [comb, S_pad], F32, kind="ExternalOutput")

    EXP = mybir.ActivationFunctionType.Exp

    with TileContext(nc) as tc:
        with (
            tc.tile_pool(name="const", bufs=1) as constp,
            tc.tile_pool(name="vg", bufs=2) as vgp,
            tc.tile_pool(name="vt", bufs=2) as vtp,
            tc.tile_pool(name="sx", bufs=2) as sxp,
            tc.tile_pool(name="pt", bufs=2) as ptp,
            tc.tile_pool(name="hot", bufs=2) as hotp,
            tc.tile_pool(name="rec", bufs=2) as recp,
            tc.tile_pool(name="ps_tsp", bufs=2, space="PSUM") as ps_tsp,
            tc.tile_pool(name="ps_s", bufs=1, space="PSUM") as ps_s,
            tc.tile_pool(name="ps_acc", bufs=1, space="PSUM") as ps_acc,
        ):
            # ---- preload constants ----
            cc_sb = constp.tile([D, NW, H, W], BF, tag="cc")
            nc.gpsimd.dma_start(cc_sb[:], cc[:].rearrange("d (n h w) -> d n h w", h=H, w=W))
            msk_sb = constp.tile([P, NW, T_w, W], BF, tag="msk")
            nc.gpsimd.dma_start(msk_sb[:], msk[:].rearrange("p (n t w) -> p n t w", t=T_w, w=W))
            gidx_sb = constp.tile([P, NW * T_w], I32, tag="gidx")
            nc.gpsimd.dma_start(gidx_sb[:], gidx[:])
            wo_sb = constp.tile([D, H, comb], BF, tag="wo")
            nc.gpsimd.dma_start(wo_sb[:], wo[:].rearrange("d (h c) -> d h c", h=H))
            bo_sb = constp.tile([comb, 1], F32, tag="bo")
            nc.gpsimd.dma_start(bo_sb[:], bo[:])
            id_sb = constp.tile([P, P], BF, tag="ident")
            nc.gpsimd.dma_start(id_sb[:], ident[:])
            ones_col = constp.tile([P, 1], BF, tag="ones_col")
            nc.vector.memset(ones_col[:], 1.0)
            ones_row = constp.tile([1, P], F32, tag="ones_row")
            nc.vector.memset(ones_row[:], 1.0)
            outb = constp.tile([comb, S_pad], F32, tag="outb")

            # scores psum: two manually-alternated halves (5 banks total)
            s_tile = ps_s.tile([P, 2, T_w, H * W], F32, tag="s")

            for w in range(NW):
                half = w % 2
                # gather this window's value rows: slot (p, t) <- tbl[gidx[p, w*T_w+t]]
                vg = vgp.tile([P, T_w, D], BF, tag="vg")
                for t in range(T_w):
                    nc.gpsimd.indirect_dma_start(
                        out=vg[:, t, :],
                        out_offset=None,
                        in_=tbl[:],
                        in_offset=IndirectOffsetOnAxis(
                            ap=gidx_sb[:, w * T_w + t:w * T_w + t + 1], axis=0
                        ),
                    )
                # V^T per tile via PE transpose
                vt = vtp.tile([P, T_w, D], BF, tag="vt")
                for t in range(T_w):
                    vps = ps_tsp.tile([P, D], BF, tag="tsp")
                    nc.tensor.transpose(vps[:], vg[:, t, :], id_sb[:])
                    nc.vector.tensor_copy(vt[:, t, :], vps[:])
                # scores: S^T[e, (h j)] = V @ C_w^T
                for t in range(T_w):
                    nc.tensor.matmul(
                        s_tile[:, half, t, :],
                        lhsT=vt[:, t, :],
                        rhs=cc_sb[:, w, :, :],
                        start=True,
                        stop=True,
                    )
                # exp then mask (mask broadcast over heads)
                sx = sxp.tile([P, T_w, H, W], BF, tag="sx")
                nc.scalar.activation(
                    sx[:].rearrange("p t h w -> p t (h w)"),
                    s_tile[:, half, :, :],
                    EXP,
                )
                pt = ptp.tile([P, T_w, H, W], BF, tag="pt")
                mv = msk_sb[:, w, :, :]  # [P, T_w, W]
                mb = AP(mv.tensor, mv.offset, [mv.ap[0], mv.ap[1], [0, H], mv.ap[2]])
                nc.vector.tensor_mul(pt[:], sx[:], mb)
                # one shared PSUM bank per window: pv group, then dn, bc, ops
                acc = ps_acc.tile([P, 2 * H * W], F32, tag="acc")
                pv_last = None
                for t in range(T_w):
                    pv_last = nc.tensor.matmul(
                        acc[:, 0:H * W],
                        lhsT=vg[:, t, :],
                        rhs=pt[:, t, :, :].rearrange("p h w -> p (h w)"),
                        start=(t == 0),
                        stop=(t == T_w - 1),
                    )
                for t in range(T_w):
                    dn_mm = nc.tensor.matmul(
                        acc[0:1, H * W:2 * H * W],
                        lhsT=ones_col[:],
                        rhs=pt[:, t, :, :].rearrange("p h w -> p (h w)"),
                        start=(t == 0),
                        stop=(t == T_w - 1),
                    )
                    if t == 0:
                        add_dep_helper(dn_mm.ins, pv_last.ins,
                                       reason="dn group after pv group (shared psum bank)")
                # reciprocal of denominators, broadcast to all partitions via K=1 matmul
                rec = recp.tile([1, H * W], F32, tag="rec")
                nc.vector.tensor_scalar_add(rec[:], acc[0:1, H * W:2 * H * W], 1e-30)
                nc.vector.reciprocal(rec[:], rec[:])
                nc.tensor.matmul(acc[:, H * W:2 * H * W], lhsT=ones_row[:], rhs=rec[:],
                                 start=True, stop=True)
                bcs = hotp.tile([P, H * W], BF, tag="bcs")
                nc.scalar.copy(bcs[:], acc[:, H * W:2 * H * W])
                hot = hotp.tile([P, H, W], BF, tag="hot")
                nc.vector.tensor_mul(
                    hot[:].rearrange("p h w -> p (h w)"), acc[:, 0:H * W], bcs[:]
                )
                # output projection: out^T[:, w] = sum_h Wo_h^T @ hot_h  (+ bo)
                for h in range(H):
                    nc.tensor.matmul(
                        acc[:, H * W:H * W + W],
                        lhsT=wo_sb[:, h, :],
                        rhs=hot[:, h, :],
                        start=(h == 0),
                        stop=(h == H - 1),
                    )
                nc.vector.tensor_scalar_add(outb[:, w * W:(w + 1) * W],
                                            acc[:, H * W:H * W + W], bo_sb[:])

            nc.sync.dma_start(out[:], outb[:])
    nc.compile()
    return nc


def _run(ast, Wq, bq, Wk, bk, Wo, bo, ast_key, ast_value, pdg_key, pdg_value, N,
         trace=False):
    """Host orchestration: build plan from data, compile, run on 8 cores."""
    n_tbl, d = ast.shape
    assert d == D
    comb = Wo.shape[1]
    E = ast_key.shape[0]
    sc = 1.0 / math.sqrt(D)

    # ---- sort edges by segment ----
    order = np.argsort(ast_value, kind="stable")
    seg_s = ast_value[order].astype(np.int64)
    key_s = ast_key[order].astype(np.int64)

    # ---- static structure ----
    S_per = -(-N // N_CORES)            # ceil
    NW = -(-S_per // W)
    S_pad = NW * W
    gw = seg_s // W                     # global window id (core = gw // NW)
    cnt = np.bincount(gw, minlength=N_CORES * NW)
    T_w = max(1, int(-(-cnt.max() // P)))

    starts = np.zeros(N_CORES * NW, np.int64)
    np.cumsum(cnt[:-1], out=starts[1:])
    rank = np.arange(E, dtype=np.int64) - starts[gw]
    slot_p = rank // T_w
    slot_t = rank % T_w
    core_of = gw // NW
    w_of = gw % NW

    gidx_all = np.zeros((N_CORES, P, NW * T_w), np.int32)
    gidx_all[core_of, slot_p, w_of * T_w + slot_t] = key_s.astype(np.int32)
    msk_f = np.zeros((N_CORES, P, NW, T_w, W), np.float32)
    msk_f[core_of, slot_p, w_of, slot_t, seg_s % W] = 1.0
    msk_all = msk_f.reshape(N_CORES, P, NW * T_w * W).astype(BF16)

    # ---- query-side fold: C = A @ (Wq' Wk^T) + bq' @ Wk^T ----
    qsrc = np.zeros(N, np.int64)
    qsrc[pdg_key.astype(np.int64)] = pdg_value.astype(np.int64)
    A = ast[qsrc]                                        # [N, D] f32
    M = np.einsum("hij,hkj->hik", Wq * sc, Wk)           # [H, D, D]
    kap = np.einsum("hj,hkj->hk", bq * sc, Wk)           # [H, D]
    C8 = np.einsum("nd,hdk->hnk", A, M) + kap[:, None, :]  # [H, N, D]
    C8p = np.zeros((H, N_CORES * S_pad, D), np.float32)
    C8p[:, :N] = C8
    cc_all = np.ascontiguousarray(
        C8p.reshape(H, N_CORES, NW, W, D).transpose(1, 4, 2, 0, 3)
    ).astype(BF16).reshape(N_CORES, D, NW * H * W)

    tblb = ast.astype(BF16)
    wo_arr = np.ascontiguousarray(
        Wo.reshape(H, D, comb).transpose(1, 0, 2)
    ).astype(BF16).reshape(D, H * comb)
    bo_col = bo.reshape(comb, 1).astype(np.float32)
    ident = np.eye(P, dtype=BF16)

    nc = _build_nc(NW, T_w, n_tbl, comb)
    in_maps = []
    for c in range(N_CORES):
        in_maps.append({
            "tbl": tblb,
            "gidx": gidx_all[c],
            "cc": cc_all[c],
            "msk": msk_all[c],
            "wo": wo_arr,
            "bo": bo_col,
            "ident": ident,
        })
    res = bass_utils.run_bass_kernel_spmd(
        nc, in_maps, core_ids=list(range(N_CORES)), trace=trace
    )
    full = np.concatenate([res.results[c]["out"].T for c in range(N_CORES)], axis=0)
    return full[:N].astype(np.float32), res


def kernel(**inputs):
    ast = np.asarray(inputs["ast_nodes_encodings"], np.float32)
    Wq = np.asarray(inputs["Wq"], np.float32)
    bq = np.asarray(inputs["bq"], np.float32)
    Wk = np.asarray(inputs["Wk"], np.float32)
    bk = np.asarray(inputs["bk"], np.float32)  # cancels inside segment softmax
    Wo = np.asarray(inputs["Wo"], np.float32)
    bo = np.asarray(inputs["bo"], np.float32)
    ast_key = np.asarray(inputs["ast_key"]).astype(np.int64)
    ast_value = np.asarray(inputs["ast_value"]).astype(np.int64)
    pdg_key = np.asarray(inputs["pdg_key"]).astype(np.int64)
    pdg_value = np.asarray(inputs["pdg_value"]).astype(np.int64)
    N = int(np.asarray(inputs["nr_cfg_nodes"]))
    out, _ = _run(ast, Wq, bq, Wk, bk, Wo, bo,
                  ast_key, ast_value, pdg_key, pdg_value, N)
    return out
